# revision 1
# baseline (speedup 1.0000x reference)
"""Bass/Trainium2 kernel for nn_BranchingGNN (bipartite GNN message passing).

Strategy (8 NeuronCores, SPMD single NEFF, per-core data differs):
  - Nodes are range-sharded: core i owns var rows [i*25000,(i+1)*25000) and
    con rows [i*12500,(i+1)*12500), padded to VR=25088 / CR=12544 rows per
    core (multiples of 128).
  - Key algebraic reformulation: messages are linear, so
        agg[d] = (sum_{e->d} h[src(e)]) @ W.T + deg(d)*b
    i.e. sum raw h rows per destination FIRST (gather + segmented sum),
    then apply the 64x64 weight in node space (12x fewer flops, and the
    gather moves raw h rows only).
  - Per direction: each core processes exactly the edges whose DESTINATION
    falls in its range. Edge sources are gathered from a replicated table
    (indirect DMA, skip-out-of-bounds padding slots), summed per dest with
    one strided DVE reduce per tile, transformed by W on the PE, combined
    with h_old + deg*b, tanh'd, and written to the core's output chunk.
  - Chunks are AllGather'd into the next direction's replicated table.
  - Per-dest slot padding: L0 capacity CAP slots per dest; dests with
    deg > CAP overflow into an L1 pre-pass whose partial sums are staged in
    extra rows appended to the gather source table, referenced by a pointer
    slot.
"""

import os
import sys
import numpy as np
from contextlib import ExitStack
from dataclasses import dataclass

sys.path.insert(0, "/opt/trn_rl_repo")

# ---------------------------------------------------------------- config

PAD_IDX = 1 << 22  # > any real table row; *64 and *256B stay in int32/uint32


@dataclass(frozen=True)
class Cfg:
    n_cores: int = 8
    nv: int = 200000          # total var nodes
    ncn: int = 100000         # total con nodes
    vf: int = 7
    cf: int = 5
    h: int = 64
    rounds: int = 2
    vr: int = 25088           # per-core var rows (mult of 512, >= nv/8)
    cr: int = 12800           # per-core con rows (mult of 512, >= ncn/8)
    cap_c: int = 16           # L0 slots per con dest (v2c direction)
    cap_v: int = 12           # L0 slots per var dest (c2v direction)
    l1_rows_c: int = 2560     # L1 rows (v2c), mult of 128
    l1_rows_v: int = 1024     # L1 rows (c2v), mult of 128
    l1_cap_c: int = 32
    l1_cap_v: int = 16
    grp_c: int = 2            # dest-tiles per gather group (v2c)
    grp_v: int = 4            # dest-tiles per gather group (c2v)

    @property
    def nvp(self):  # rows in the AllGather'd var table
        return self.n_cores * self.vr

    @property
    def ncp(self):
        return self.n_cores * self.cr

    @property
    def v_own(self):  # real rows owned per core
        return self.nv // self.n_cores

    @property
    def c_own(self):
        return self.ncn // self.n_cores


FULL = Cfg()

# ---------------------------------------------------------------- host prep


def _remap(ids, own, rows):
    """global node id -> padded table row id"""
    return (ids // own) * rows + (ids % own)


def _build_slots(src_rows, dst_local, n_dst, cap, l1_rows, l1_cap, stg_base):
    """Build L0 [n_dst, cap] and L1 [l1_rows, l1_cap] int32 index arrays for
    one core's one direction.  src_rows: table row of each edge's source;
    dst_local: local dest row in [0, n_dst); both length = n_edges_core.
    Returns (l0, l1, deg) with PAD_IDX padding; dests with deg > cap place
    their first cap-1 edges in L0, a pointer (stg_base + l1_row) in slot
    cap-1, and the rest in their L1 row."""
    order = np.argsort(dst_local, kind="stable")
    d = dst_local[order]
    s = src_rows[order]
    deg = np.bincount(d, minlength=n_dst).astype(np.int64)
    start = np.concatenate([[0], np.cumsum(deg)[:-1]])
    slot = np.arange(len(d)) - start[d]  # rank of edge within its dest

    l0 = np.full((n_dst, cap), PAD_IDX, np.int32)
    l1 = np.full((l1_rows, l1_cap), PAD_IDX, np.int32)

    big = deg > cap  # dests needing an L1 row
    n_big = int(big.sum())
    assert n_big <= l1_rows, (n_big, l1_rows)
    assert deg.max(initial=0) <= (cap - 1) + l1_cap, deg.max()
    l1_of = np.full(n_dst, -1, np.int64)
    l1_of[big] = np.arange(n_big)

    is_big_e = big[d]
    # small dests: all edges in L0. big dests: slots 0..cap-2 in L0.
    in_l0 = (~is_big_e & (slot < cap)) | (is_big_e & (slot < cap - 1))
    l0[d[in_l0], slot[in_l0]] = s[in_l0]
    # pointer slots
    l0[np.where(big)[0], cap - 1] = stg_base + l1_of[big]
    # overflow edges
    ov = is_big_e & (slot >= cap - 1)
    l1[l1_of[d[ov]], slot[ov] - (cap - 1)] = s[ov]
    return l0, l1, deg.astype(np.float32)


def prep_inputs(inputs, cfg: Cfg):
    """Full numpy preprocessing -> list of per-core input dicts."""
    c = cfg
    ev = np.asarray(inputs["edge_var"])
    ec = np.asarray(inputs["edge_con"])
    xv = np.asarray(inputs["var_features"], np.float32)
    xc = np.asarray(inputs["con_features"], np.float32)

    # padded, transposed feature arrays
    xv_t = np.zeros((c.vf, c.nvp), np.float32)
    xc_t = np.zeros((c.cf, c.ncp), np.float32)
    vrow = _remap(np.arange(c.nv), c.v_own, c.vr)
    crow = _remap(np.arange(c.ncn), c.c_own, c.cr)
    xv_t[:, vrow] = xv.T
    xc_t[:, crow] = xc.T

    ev_row = _remap(ev, c.v_own, c.vr).astype(np.int64)
    ec_row = _remap(ec, c.c_own, c.cr).astype(np.int64)

    per_core = []
    for i in range(c.n_cores):
        m_c = (ec // c.c_own) == i  # edges whose con-dest is on core i
        m_v = (ev // c.v_own) == i
        l0c, l1c, degc = _build_slots(
            ev_row[m_c], (ec[m_c] % c.c_own).astype(np.int64), c.cr,
            c.cap_c, c.l1_rows_c, c.l1_cap_c, c.nvp)
        l0v, l1v, degv = _build_slots(
            ec_row[m_v], (ev[m_v] % c.v_own).astype(np.int64), c.vr,
            c.cap_v, c.l1_rows_v, c.l1_cap_v, c.ncp)
        per_core.append(dict(
            xv_t=np.ascontiguousarray(xv_t[:, i * c.vr:(i + 1) * c.vr]),
            xc_t=np.ascontiguousarray(xc_t[:, i * c.cr:(i + 1) * c.cr]),
            idx_v2c_l0=l0c, idx_v2c_l1=l1c, deg_con=degc,
            idx_c2v_l0=l0v, idx_c2v_l1=l1v, deg_var=degv,
        ))

    # weights (shared across cores)
    w = {}
    w["w1v_t"] = np.ascontiguousarray(np.asarray(inputs["W_ve1"], np.float32).T)  # [vf,64]
    w["w2v_t"] = np.ascontiguousarray(np.asarray(inputs["W_ve2"], np.float32).T)  # [64,64]
    w["b1v"] = np.asarray(inputs["b_ve1"], np.float32).reshape(c.h, 1)
    w["b2v"] = np.asarray(inputs["b_ve2"], np.float32).reshape(c.h, 1)
    w["w1c_t"] = np.ascontiguousarray(np.asarray(inputs["W_ce1"], np.float32).T)
    w["w2c_t"] = np.ascontiguousarray(np.asarray(inputs["W_ce2"], np.float32).T)
    w["b1c"] = np.asarray(inputs["b_ce1"], np.float32).reshape(c.h, 1)
    w["b2c"] = np.asarray(inputs["b_ce2"], np.float32).reshape(c.h, 1)
    for r in range(c.rounds):
        w[f"wt_v2c_{r}"] = np.ascontiguousarray(
            np.asarray(inputs["W_v2c"], np.float32)[r].T)  # [64,64] = W.T
        w[f"wt_c2v_{r}"] = np.ascontiguousarray(
            np.asarray(inputs["W_c2v"], np.float32)[r].T)
        w[f"b_v2c_{r}"] = np.broadcast_to(
            np.asarray(inputs["b_v2c"], np.float32)[r], (128, c.h)).copy()
        w[f"b_c2v_{r}"] = np.broadcast_to(
            np.asarray(inputs["b_c2v"], np.float32)[r], (128, c.h)).copy()
    w["wro_rep"] = np.ascontiguousarray(np.tile(
        np.asarray(inputs["W_ro"], np.float32).reshape(1, c.h), (128, c.grp_v)))
    w["b_ro"] = float(np.asarray(inputs["b_ro"]).reshape(())[()]) \
        if np.asarray(inputs["b_ro"]).size == 1 else float(inputs["b_ro"][0])

    for pc in per_core:
        pc.update({k: v for k, v in w.items() if not isinstance(v, float)})
    return per_core, w["b_ro"]


# ---------------------------------------------------------------- builder

def build_nc(cfg: Cfg, b_ro: float):
    from concourse import bass, mybir, tile
    import concourse.bacc as bacc
    from concourse.masks import make_identity

    c = cfg
    f32 = mybir.dt.float32
    i32 = mybir.dt.int32
    H = c.h

    nc = bacc.Bacc("TRN2", target_bir_lowering=False, debug=False,
                   num_devices=c.n_cores)

    # ---- I/O tensors
    def inp(name, shape, dt=f32):
        return nc.dram_tensor(name, list(shape), dt, kind="ExternalInput").ap()

    xv_t = inp("xv_t", [c.vf, c.vr])
    xc_t = inp("xc_t", [c.cf, c.cr])
    idx_v2c_l0 = inp("idx_v2c_l0", [c.cr, c.cap_c], i32)
    idx_v2c_l1 = inp("idx_v2c_l1", [c.l1_rows_c, c.l1_cap_c], i32)
    deg_con = inp("deg_con", [c.cr])
    idx_c2v_l0 = inp("idx_c2v_l0", [c.vr, c.cap_v], i32)
    idx_c2v_l1 = inp("idx_c2v_l1", [c.l1_rows_v, c.l1_cap_v], i32)
    deg_var = inp("deg_var", [c.vr])
    w1v_t = inp("w1v_t", [c.vf, H]); w2v_t = inp("w2v_t", [H, H])
    b1v = inp("b1v", [H, 1]); b2v = inp("b2v", [H, 1])
    w1c_t = inp("w1c_t", [c.cf, H]); w2c_t = inp("w2c_t", [H, H])
    b1c = inp("b1c", [H, 1]); b2c = inp("b2c", [H, 1])
    wts = {}
    for r in range(c.rounds):
        wts[("v2c", r)] = (inp(f"wt_v2c_{r}", [H, H]), inp(f"b_v2c_{r}", [128, H]))
        wts[("c2v", r)] = (inp(f"wt_c2v_{r}", [H, H]), inp(f"b_c2v_{r}", [128, H]))
    wro_rep = inp("wro_rep", [128, c.grp_v * H])
    scores = nc.dram_tensor("scores", [c.vr], f32, kind="ExternalOutput").ap()

    groups = [list(range(c.n_cores))]

    with tile.TileContext(nc) as tc:
        with ExitStack() as ctx:
            dram = ctx.enter_context(tc.tile_pool(name="dram", bufs=1, space="DRAM"))
            cpool = ctx.enter_context(tc.tile_pool(name="consts", bufs=1))
            sb = ctx.enter_context(tc.tile_pool(name="sb", bufs=3))
            sb2 = ctx.enter_context(tc.tile_pool(name="sb2", bufs=2))
            ps = ctx.enter_context(tc.tile_pool(name="ps", bufs=2, space="PSUM"))

            # tables & chunks (DRAM)
            var_tab = [dram.tile([c.nvp + c.l1_rows_c, H], f32,
                                 name=f"var_tab{r}", tag=f"var_tab{r}")
                       for r in range(c.rounds)]
            con_tab = [dram.tile([c.ncp + c.l1_rows_v, H], f32,
                                 name=f"con_tab{r}", tag=f"con_tab{r}")
                       for r in range(c.rounds)]
            chunk_var = [dram.tile([c.vr, H], f32, name=f"chunk_var{j}",
                                   tag=f"chunk_var{j}") for j in range(2)]
            chunk_con = [dram.tile([c.cr, H], f32, name=f"chunk_con{j}",
                                   tag=f"chunk_con{j}") for j in range(3)]

            ident = cpool.tile([128, 128], f32, name="ident", tag="ident")
            make_identity(nc, ident)

            # ---------------- encoder: x_t [F, rows] -> chunk [rows, H]
            def encode(x_t, F, rows, w1, b1, w2, b2, out_chunk):
                w1_sb = sb2.tile([F, H], f32, name="w1_sb", tag="encw1")
                nc.sync.dma_start(w1_sb[:], w1[:])
                w2_sb = sb2.tile([H, H], f32, name="w2_sb", tag="encw2")
                nc.sync.dma_start(w2_sb[:], w2[:])
                b1_sb = sb2.tile([H, 1], f32, name="b1_sb", tag="encb1")
                nc.sync.dma_start(b1_sb[:], b1[:])
                b2_sb = sb2.tile([H, 1], f32, name="b2_sb", tag="encb2")
                nc.sync.dma_start(b2_sb[:], b2[:])
                for t in range(rows // 512):
                    xt = sb.tile([F, 512], f32, name="xt", tag="enc_xt")
                    nc.sync.dma_start(xt[:], x_t[:, t * 512:(t + 1) * 512])
                    p1 = ps.tile([H, 512], f32, name="p1", tag="mmA")
                    nc.tensor.matmul(p1[:], lhsT=w1_sb[:], rhs=xt[:],
                                     start=True, stop=True)
                    t1 = sb.tile([H, 512], f32, name="t1", tag="enc_t1")
                    nc.scalar.activation(t1[:], p1[:],
                                         mybir.ActivationFunctionType.Tanh,
                                         bias=b1_sb[:, :])
                    p2 = ps.tile([H, 512], f32, name="p2", tag="mmA")
                    nc.tensor.matmul(p2[:], lhsT=w2_sb[:], rhs=t1[:],
                                     start=True, stop=True)
                    h2 = sb.tile([H, 512], f32, name="h2", tag="enc_h2")
                    nc.vector.tensor_scalar_add(h2[:], p2[:], b2_sb[:, :])
                    hn = sb.tile([128, 4 * H], f32, name="hn", tag="enc_hn")
                    for q in range(4):
                        pt = ps.tile([128, H], f32, name="pt", tag="trp")
                        nc.tensor.transpose(
                            pt[:], h2[:, q * 128:(q + 1) * 128], ident[:H, :H])
                        nc.scalar.activation(
                            hn[:, q * H:(q + 1) * H], pt[:],
                            mybir.ActivationFunctionType.Copy)
                    # store 512 rows; row r=t*512+q*128+p -> hn[p, q*H:...]
                    nc.sync.dma_start(
                        out_chunk[t * 512:(t + 1) * 512, :].rearrange(
                            "(q p) f -> p q f", p=128), hn[:])

            encode(xv_t, c.vf, c.vr, w1v_t, b1v, w2v_t, b2v, chunk_var[0])
            encode(xc_t, c.cf, c.cr, w1c_t, b1c, w2c_t, b2c, chunk_con[0])

            def allgather(chunk, tab, rows_total):
                nc.gpsimd.collective_compute(
                    "AllGather", mybir.AluOpType.bypass,
                    replica_groups=groups,
                    ins=[chunk[:, :]],
                    outs=[tab[0:rows_total, :]],
                )

            allgather(chunk_var[0], var_tab[0], c.nvp)

            # ---------------- one message-passing direction
            def msg_pass(src_tab, src_rows_total, l1_idx, l1_rows, l1_cap,
                         l0_idx, n_dst, cap, grp, deg, w_t, b_rep,
                         h_old_chunk, out_chunk, readout=None):
                bound_l1 = src_rows_total - 1        # L1 reads real rows only
                bound = src_rows_total + l1_rows - 1  # L0 may read staging too
                src_real = src_tab[0:src_rows_total, :]
                src_all = src_tab[0:src_rows_total + l1_rows, :]
                wt_sb = sb2.tile([H, H], f32, name="wt_sb", tag="msg_wt")
                nc.sync.dma_start(wt_sb[:], w_t[:])
                brep_sb = sb2.tile([128, H], f32, name="brep_sb", tag="msg_brep")
                nc.sync.dma_start(brep_sb[:], b_rep[:])

                # L1 pre-pass: staged partial sums for high-degree dests
                for t in range(l1_rows // 128):
                    lbuf = sb.tile([128, l1_cap * H], f32, name="lbuf", tag="gbuf")
                    nc.vector.memset(lbuf[:], 0.0)
                    lidx = sb.tile([128, l1_cap], i32, name="lidx", tag="gidx")
                    nc.sync.dma_start(
                        lidx[:], l1_idx[t * 128:(t + 1) * 128, :])
                    for s in range(l1_cap):
                        nc.gpsimd.indirect_dma_start(
                            out=lbuf[:, s * H:(s + 1) * H], out_offset=None,
                            in_=src_real,
                            in_offset=bass.IndirectOffsetOnAxis(
                                ap=lidx[:, s:s + 1], axis=0),
                            bounds_check=bound_l1, oob_is_err=False)
                    part = sb.tile([128, H], f32, name="part", tag="l1part")
                    nc.vector.tensor_reduce(
                        part[:],
                        lbuf[:].rearrange("p (s f) -> p f s", f=H),
                        axis=mybir.AxisListType.X, op=mybir.AluOpType.add)
                    nc.sync.dma_start(
                        src_tab[src_rows_total + t * 128:
                                src_rows_total + (t + 1) * 128, :], part[:])

                # L0 phase, grp dest-tiles at a time
                ntiles = n_dst // 128
                assert ntiles % grp == 0
                for g0 in range(ntiles // grp):
                    t0 = g0 * grp
                    gbuf = sb.tile([128, grp * cap * H], f32, name="gbuf",
                                   tag="gbuf")
                    nc.vector.memset(gbuf[:], 0.0)
                    gidx = sb.tile([128, grp * cap], i32, name="gidx", tag="gidx")
                    # l0_idx rows (t*128+p) slot s -> gidx[p, t*cap+s]
                    nc.sync.dma_start(
                        gidx[:],
                        l0_idx[t0 * 128:(t0 + grp) * 128, :].rearrange(
                            "(t p) s -> p t s", p=128))
                    for col in range(grp * cap):
                        nc.gpsimd.indirect_dma_start(
                            out=gbuf[:, col * H:(col + 1) * H], out_offset=None,
                            in_=src_all,
                            in_offset=bass.IndirectOffsetOnAxis(
                                ap=gidx[:, col:col + 1], axis=0),
                            bounds_check=bound, oob_is_err=False)
                    G = sb.tile([128, grp * H], f32, name="G", tag="Gsum")
                    nc.vector.tensor_reduce(
                        G[:].rearrange("p (t f) -> p t f", f=H),
                        gbuf[:].rearrange("p (t s f) -> p t f s", s=cap, f=H),
                        axis=mybir.AxisListType.X, op=mybir.AluOpType.add)
                    hold = sb.tile([128, grp * H], f32, name="hold", tag="hold")
                    nc.sync.dma_start(
                        hold[:],
                        h_old_chunk[t0 * 128:(t0 + grp) * 128, :].rearrange(
                            "(t p) f -> p t f", p=128))
                    degc = sb.tile([128, grp], f32, name="degc", tag="degc")
                    nc.sync.dma_start(
                        degc[:],
                        deg[t0 * 128:(t0 + grp) * 128].rearrange(
                            "(t p) -> p t", p=128))
                    hnew = sb.tile([128, grp * H], f32, name="hnew", tag="hnew")
                    for k in range(grp):
                        gk = G[:, k * H:(k + 1) * H]
                        # G feat-major for the W matmul
                        ptr = ps.tile([H, 128], f32, name="ptr", tag="trp")
                        nc.tensor.transpose(ptr[:], gk, ident[:, :])
                        gfm = sb.tile([H, 128], f32, name="gfm", tag="gfm")
                        nc.scalar.activation(
                            gfm[:], ptr[:], mybir.ActivationFunctionType.Copy)
                        agg = ps.tile([128, H], f32, name="agg", tag="agg")
                        nc.tensor.matmul(agg[:], lhsT=gfm[:], rhs=wt_sb[:],
                                         start=True, stop=True)
                        hk = hnew[:, k * H:(k + 1) * H]
                        # hk = h_old + deg*b + agg, then tanh
                        nc.vector.tensor_scalar_mul(
                            hk, brep_sb[:], degc[:, k:k + 1])
                        nc.vector.tensor_add(
                            hk, hk, hold[:, k * H:(k + 1) * H])
                        nc.vector.tensor_add(hk, hk, agg[:])
                        nc.scalar.activation(
                            hk, hk, mybir.ActivationFunctionType.Tanh)
                    if out_chunk is not None:
                        nc.sync.dma_start(
                            out_chunk[t0 * 128:(t0 + grp) * 128, :].rearrange(
                                "(t p) f -> p t f", p=128), hnew[:])
                    if readout is not None:
                        wro_sb, sc_sb = readout
                        m = sb.tile([128, grp * H], f32, name="m", tag="romul")
                        nc.vector.tensor_mul(m[:], hnew[:], wro_sb[:])
                        nc.vector.tensor_reduce(
                            sc_sb[:, t0:t0 + grp],
                            m[:].rearrange("p (t f) -> p t f", f=H),
                            axis=mybir.AxisListType.X, op=mybir.AluOpType.add)

            seq = []
            for r in range(c.rounds):
                seq.append(("v2c", r))
                seq.append(("c2v", r))

            wro_sb = cpool.tile([128, c.grp_v * H], f32, name="wro_sb",
                                tag="wro_sb")
            nc.sync.dma_start(wro_sb[:], wro_rep[:])
            sc_sb = cpool.tile([128, c.vr // 128], f32, name="sc_sb",
                               tag="sc_sb")

            con_state = chunk_con[0]
            var_state = chunk_var[0]
            for (d, r) in seq:
                last = (d, r) == seq[-1]
                w_t, b_rep = wts[(d, r)]
                if d == "v2c":
                    out = chunk_con[r + 1]
                    msg_pass(var_tab[r], c.nvp, idx_v2c_l1, c.l1_rows_c,
                             c.l1_cap_c, idx_v2c_l0, c.cr, c.cap_c, c.grp_c,
                             deg_con, w_t, b_rep, con_state, out)
                    allgather(out, con_tab[r], c.ncp)
                    con_state = out
                else:
                    out = None if last else chunk_var[r + 1]
                    msg_pass(con_tab[r], c.ncp, idx_c2v_l1, c.l1_rows_v,
                             c.l1_cap_v, idx_c2v_l0, c.vr, c.cap_v, c.grp_v,
                             deg_var, w_t, b_rep, var_state, out,
                             readout=(wro_sb, sc_sb) if last else None)
                    if not last:
                        allgather(out, var_tab[r + 1], c.nvp)
                        var_state = out

            # readout epilogue: sc_sb [128, ntiles] -> scores [vr]
            nt = c.vr // 128
            nc.vector.tensor_scalar_add(sc_sb[:], sc_sb[:], float(b_ro))
            for half in range(2):
                w2 = nt // 2
                pt = ps.tile([w2, 128], f32, name="pt_ro", tag="trp")
                nc.tensor.transpose(
                    pt[:], sc_sb[:, half * w2:(half + 1) * w2], ident[:, :])
                so = sb.tile([w2, 128], f32, name="so", tag="so")
                nc.scalar.activation(
                    so[:], pt[:], mybir.ActivationFunctionType.Copy)
                nc.sync.dma_start(
                    scores[half * w2 * 128:(half + 1) * w2 * 128].rearrange(
                        "(q p) -> q p", p=128), so[:])

    nc.compile()
    return nc


# ---------------------------------------------------------------- runner

_CACHE = {}


def _get_nc(cfg, b_ro):
    key = (cfg, round(b_ro, 10))
    if key not in _CACHE:
        _CACHE[key] = build_nc(cfg, b_ro)
    return _CACHE[key]


def run(inputs, cfg: Cfg = FULL, trace=False):
    from concourse import bass_utils
    per_core, b_ro = prep_inputs(inputs, cfg)
    nc = _get_nc(cfg, b_ro)
    res = bass_utils.run_bass_kernel_spmd(
        nc, per_core, core_ids=list(range(cfg.n_cores)), trace=trace)
    out = np.concatenate([r["scores"][:cfg.v_own] for r in res.results])
    return out.astype(np.float32), res


def kernel(**inputs) -> np.ndarray:
    out, _ = run(inputs, FULL)
    return out



# revision 26
# speedup vs baseline: 1.0960x; 1.0960x over previous
"""Bass/Trainium2 kernel for nn_BranchingGNN (bipartite GNN message passing).

Strategy (8 NeuronCores, SPMD single NEFF, per-core data differs):
  - Nodes range-sharded: core i owns var rows [i*25000,(i+1)*25000) and con
    rows [i*12500,(i+1)*12500), padded to VR=25088 / CR=12800 rows per core.
    Within a core, rows are split into NRANGE=4 dest ranges, each ending in
    its own trash rows (real rows per range: 6250 var / 3125 con), so dummy
    padding edges can scatter into in-range trash.
  - Messages are linear:  agg[dst] = (sum_{e->dst} h[src(e)]) @ W.T + deg*b,
    so we aggregate RAW h rows per dest first, then apply the 64x64 W once
    per node.
  - Per direction, each core processes edges whose DEST it owns:
      * SWDGE dma_gather: per-edge h_src rows (256B) from the replicated
        src table (chunked <=32768 rows for int16 indices) into SBUF
        positional buffers  (~0.34ns/descriptor on Pool engine).
      * SWDGE dma_scatter_add: positional rows += into a local per-core
        HBM accumulator at the edge's dest row.
      * Edges are bucketed by (dest range, src chunk); cell sizes are
        max'd across cores and padded with dummy edges (gather chunk row
        0, scatter to the range's trash row) so the SPMD instruction
        stream is core-uniform.
      * Per dest range, once its scatters land: read acc tiles, transpose
        (PE), apply W (PE matmul), add h_old + deg*b, tanh, write the new
        state chunk (overlaps later ranges' gather/scatter DMA).
  - State chunks are AllGather'd into the next direction's replicated table.
  - Readout fused into the last direction's tile pipeline.
"""

import sys
import numpy as np
from contextlib import ExitStack
from dataclasses import dataclass

sys.path.insert(0, "/opt/trn_rl_repo")

# ---------------------------------------------------------------- config


@dataclass(frozen=True)
class Cfg:
    n_cores: int = 8
    nv: int = 200000
    ncn: int = 100000
    vf: int = 7
    cf: int = 5
    h: int = 64
    rounds: int = 2
    vr: int = 25088           # per-core var rows (49*512)
    cr: int = 12800           # per-core con rows (25*512)
    nrange: int = 4           # dest ranges per core
    chunk: int = 32768        # gather src chunk rows (int16 index bound)
    max_cell: int = 16384     # max edges per gather/scatter instruction

    @property
    def nvp(self):
        return self.n_cores * self.vr

    @property
    def ncp(self):
        return self.n_cores * self.cr

    @property
    def v_own(self):
        return self.nv // self.n_cores      # 25000

    @property
    def c_own(self):
        return self.ncn // self.n_cores     # 12500

    @property
    def vrange(self):
        return self.vr // self.nrange       # 6272

    @property
    def crange(self):
        return self.cr // self.nrange       # 3200

    @property
    def v_real_r(self):
        return self.v_own // self.nrange    # 6250

    @property
    def c_real_r(self):
        return self.c_own // self.nrange    # 3125

    @property
    def nch_v(self):  # src chunks in the var table
        return -(-self.nvp // self.chunk)   # 7

    @property
    def nch_c(self):
        return -(-self.ncp // self.chunk)   # 4


FULL = Cfg()

# ---------------------------------------------------------------- host prep


def _node_row(g, own, rows, real_r, rng):
    """global node id -> padded+permuted global table row"""
    core = g // own
    loc = g % own
    return core * rows + (loc // real_r) * rng + (loc % real_r)


def _rank_within_dest(s):
    """Occurrence rank of each element of sorted dest array s."""
    if len(s) == 0:
        return np.zeros(0, np.int64)
    uniq, cnt = np.unique(s, return_counts=True)
    starts = np.concatenate([[0], np.cumsum(cnt)[:-1]])
    return np.arange(len(s)) - np.repeat(starts, cnt)


def _build_dir(src_glob_row, dst_range, dst_off, trash, cell_waves, c: Cfg):
    """Build one core's one direction: int16 gather/scatter idx streams.
    Cells in sorted (range, chunk) order; within a cell, waves of
    unique-dest edges (rank w within dest) concatenated, each padded to
    its uniform size with dummies (gather row 0, scatter trash row)."""
    gidx_parts, sidx_parts = [], []
    for (k, ch), waves in cell_waves.items():
        m = (dst_range == k) & (src_glob_row // c.chunk == ch)
        g_all = (src_glob_row[m] % c.chunk).astype(np.int64)
        s_all = dst_off[m].astype(np.int64)
        o = np.argsort(s_all, kind="stable")
        g_all, s_all = g_all[o], s_all[o]
        rank = _rank_within_dest(s_all)
        for w, size in enumerate(waves):
            sel = rank == w
            g, s = g_all[sel], s_all[sel]
            pad = size - len(g)
            assert pad >= 0, (k, ch, w, size, len(g))
            gidx_parts.append(np.concatenate([g, np.zeros(pad, np.int64)]))
            sidx_parts.append(np.concatenate(
                [s, np.full(pad, trash, np.int64)]))
    gidx = np.concatenate(gidx_parts) if gidx_parts else np.zeros(0, np.int64)
    sidx = np.concatenate(sidx_parts) if sidx_parts else np.zeros(0, np.int64)
    assert gidx.max(initial=0) < c.chunk and sidx.max(initial=0) < 32768

    def wrap(a):  # position j -> [j%16, j//16], tiled to 128 partitions
        a16 = a.astype(np.int16).reshape(-1, 16).T
        return np.ascontiguousarray(np.tile(a16, (8, 1)))

    return wrap(gidx), wrap(sidx)


def _cell_plan(per_core_dat, c: Cfg, nrange, nchunk):
    """Uniform wave structure: per (range, chunk) cell, per-wave sizes are
    the max count across cores, rounded up to 128.  Returns
    {(k, ch): (w0_size, w1_size, ...)}."""
    per_core_counts = []  # core -> {(k,ch): np.bincount of ranks}
    for (g, k, off) in per_core_dat:
        ch = g // c.chunk
        d = {}
        for kk in range(nrange):
            for cc in range(nchunk):
                sel = (k == kk) & (ch == cc)
                if not sel.any():
                    continue
                s = np.sort(off[sel])
                rank = _rank_within_dest(s)
                d[(kk, cc)] = np.bincount(rank)
        per_core_counts.append(d)
    plan = {}
    for kk in range(nrange):
        for cc in range(nchunk):
            maxw = max((len(d.get((kk, cc), ())) for d in per_core_counts),
                       default=0)
            if maxw == 0:
                continue
            waves = []
            for w in range(maxw):
                mx = max(int(d[(kk, cc)][w])
                         if (kk, cc) in d and w < len(d[(kk, cc)]) else 0
                         for d in per_core_counts)
                sz = -(-mx // 128) * 128
                assert sz <= c.max_cell
                waves.append(sz)
            plan[(kk, cc)] = tuple(waves)
    return plan


def prep_inputs(inputs, cfg: Cfg = FULL):
    c = cfg
    ev = np.asarray(inputs["edge_var"]).astype(np.int64)
    ec = np.asarray(inputs["edge_con"]).astype(np.int64)
    xv = np.asarray(inputs["var_features"], np.float32)
    xc = np.asarray(inputs["con_features"], np.float32)

    # permuted in-core row of every node
    v_loc = ev % c.v_own
    c_loc = ec % c.c_own
    ev_grow = _node_row(ev, c.v_own, c.vr, c.v_real_r, c.vrange)
    ec_grow = _node_row(ec, c.c_own, c.cr, c.c_real_r, c.crange)

    # ---- per-core edge sets + cell keys
    v2c_dat, c2v_dat = [], []
    for i in range(c.n_cores):
        m = (ec // c.c_own) == i          # v2c: dest con on core i
        k = (c_loc[m] // c.c_real_r)
        off = c_loc[m] % c.c_real_r
        v2c_dat.append((ev_grow[m], k, off))
        m = (ev // c.v_own) == i          # c2v: dest var on core i
        k = (v_loc[m] // c.v_real_r)
        off = v_loc[m] % c.v_real_r
        c2v_dat.append((ec_grow[m], k, off))

    cells_v2c = _cell_plan(v2c_dat, c, c.nrange, c.nch_v)
    cells_c2v = _cell_plan(c2v_dat, c, c.nrange, c.nch_c)

    # ---- per-core tensors
    # features, transposed + permuted
    all_v = np.arange(c.nv)
    all_c = np.arange(c.ncn)
    vrow = _node_row(all_v, c.v_own, c.vr, c.v_real_r, c.vrange)
    crow = _node_row(all_c, c.c_own, c.cr, c.c_real_r, c.crange)
    xv_t = np.zeros((c.vf, c.nvp), np.float32)
    xc_t = np.zeros((c.cf, c.ncp), np.float32)
    xv_t[:, vrow] = xv.T
    xc_t[:, crow] = xc.T
    deg_con_full = np.zeros(c.ncp, np.float32)
    np.add.at(deg_con_full, ec_grow, 1.0)
    deg_var_full = np.zeros(c.nvp, np.float32)
    np.add.at(deg_var_full, ev_grow, 1.0)

    per_core = []
    for i in range(c.n_cores):
        g, k, off = v2c_dat[i]
        gi_v2c, si_v2c = _build_dir(g, k, off, c.crange - 1, cells_v2c, c)
        g, k, off = c2v_dat[i]
        gi_c2v, si_c2v = _build_dir(g, k, off, c.vrange - 1, cells_c2v, c)
        per_core.append(dict(
            xv_t=np.ascontiguousarray(xv_t[:, i * c.vr:(i + 1) * c.vr]),
            xc_t=np.ascontiguousarray(xc_t[:, i * c.cr:(i + 1) * c.cr]),
            gidx_v2c=gi_v2c, sidx_v2c=si_v2c,
            gidx_c2v=gi_c2v, sidx_c2v=si_c2v,
            deg_con=np.ascontiguousarray(
                deg_con_full[i * c.cr:(i + 1) * c.cr]),
            deg_var=np.ascontiguousarray(
                deg_var_full[i * c.vr:(i + 1) * c.vr]),
        ))

    # ---- weights (replicated)
    w = {}
    w["w1v_t"] = np.ascontiguousarray(np.asarray(inputs["W_ve1"], np.float32).T)
    w["w2v_t"] = np.ascontiguousarray(np.asarray(inputs["W_ve2"], np.float32).T)
    w["b1v"] = np.asarray(inputs["b_ve1"], np.float32).reshape(c.h, 1)
    w["b2v"] = np.asarray(inputs["b_ve2"], np.float32).reshape(c.h, 1)
    w["w1c_t"] = np.ascontiguousarray(np.asarray(inputs["W_ce1"], np.float32).T)
    w["w2c_t"] = np.ascontiguousarray(np.asarray(inputs["W_ce2"], np.float32).T)
    w["b1c"] = np.asarray(inputs["b_ce1"], np.float32).reshape(c.h, 1)
    w["b2c"] = np.asarray(inputs["b_ce2"], np.float32).reshape(c.h, 1)
    for r in range(c.rounds):
        w[f"wt_v2c_{r}"] = np.ascontiguousarray(
            np.asarray(inputs["W_v2c"], np.float32)[r].T)
        w[f"wt_c2v_{r}"] = np.ascontiguousarray(
            np.asarray(inputs["W_c2v"], np.float32)[r].T)
        w[f"b_v2c_{r}"] = np.broadcast_to(
            np.asarray(inputs["b_v2c"], np.float32)[r], (128, c.h)).copy()
        w[f"b_c2v_{r}"] = np.broadcast_to(
            np.asarray(inputs["b_c2v"], np.float32)[r], (128, c.h)).copy()
    w["wro_rep"] = np.ascontiguousarray(np.broadcast_to(
        np.asarray(inputs["W_ro"], np.float32).reshape(1, c.h),
        (128, c.h)).copy())
    b_ro = float(np.asarray(inputs["b_ro"]).reshape(-1)[0])

    for pc in per_core:
        pc.update(w)

    meta = (tuple(sorted(cells_v2c.items())),
            tuple(sorted(cells_c2v.items())), b_ro)
    return per_core, meta


def extract_scores(results, cfg: Cfg = FULL):
    """Per-core permuted scores [vr] -> full unpermuted [nv]."""
    c = cfg
    out = np.empty(c.nv, np.float32)
    for i, r in enumerate(results):
        s = np.asarray(r["scores"])
        for k in range(c.nrange):
            out[i * c.v_own + k * c.v_real_r:
                i * c.v_own + (k + 1) * c.v_real_r] = \
                s[k * c.vrange: k * c.vrange + c.v_real_r]
    return out


# ---------------------------------------------------------------- builder

def build_nc(cfg: Cfg, meta):
    import os
    from concourse import bass, mybir, tile
    import concourse.bacc as bacc
    from concourse.masks import make_identity

    # debug: truncate the kernel after N phases (8 = full)
    trunc = int(os.environ.get("BGNN_TRUNC", "99"))

    cells_v2c = dict(meta[0])
    cells_c2v = dict(meta[1])
    b_ro = meta[2]
    c = cfg
    f32 = mybir.dt.float32
    i16 = mybir.dt.int16
    H = c.h
    NACC = 3
    E_v2c = sum(sum(w) for w in cells_v2c.values())
    E_c2v = sum(sum(w) for w in cells_c2v.values())
    B = max(max(sum(w) for w in cells_v2c.values()),
            max(sum(w) for w in cells_c2v.values()))

    nc = bacc.Bacc("TRN2", target_bir_lowering=False, debug=False,
                   num_devices=c.n_cores)

    def inp(name, shape, dt=f32):
        return nc.dram_tensor(name, list(shape), dt, kind="ExternalInput").ap()

    xv_t = inp("xv_t", [c.vf, c.vr])
    xc_t = inp("xc_t", [c.cf, c.cr])
    gidx_v2c = inp("gidx_v2c", [128, E_v2c // 16], i16)
    sidx_v2c = inp("sidx_v2c", [128, E_v2c // 16], i16)
    gidx_c2v = inp("gidx_c2v", [128, E_c2v // 16], i16)
    sidx_c2v = inp("sidx_c2v", [128, E_c2v // 16], i16)
    deg_con = inp("deg_con", [c.cr])
    deg_var = inp("deg_var", [c.vr])
    w1v_t = inp("w1v_t", [c.vf, H]); w2v_t = inp("w2v_t", [H, H])
    b1v = inp("b1v", [H, 1]); b2v = inp("b2v", [H, 1])
    w1c_t = inp("w1c_t", [c.cf, H]); w2c_t = inp("w2c_t", [H, H])
    b1c = inp("b1c", [H, 1]); b2c = inp("b2c", [H, 1])
    wts = {}
    for r in range(c.rounds):
        wts[("v2c", r)] = (inp(f"wt_v2c_{r}", [H, H]),
                           inp(f"b_v2c_{r}", [128, H]))
        wts[("c2v", r)] = (inp(f"wt_c2v_{r}", [H, H]),
                           inp(f"b_c2v_{r}", [128, H]))
    wro_rep = inp("wro_rep", [128, H])
    scores = nc.dram_tensor("scores", [c.vr], f32, kind="ExternalOutput").ap()

    groups = [list(range(c.n_cores))]

    with tile.TileContext(nc) as tc:
        with ExitStack() as ctx:
            dram = ctx.enter_context(
                tc.tile_pool(name="dram", bufs=1, space="DRAM"))
            cpool = ctx.enter_context(tc.tile_pool(name="consts", bufs=1))
            sb = ctx.enter_context(tc.tile_pool(name="sb", bufs=3))
            sb2 = ctx.enter_context(tc.tile_pool(name="sb2", bufs=2))
            gpool = ctx.enter_context(tc.tile_pool(name="gpool", bufs=3))
            ipool = ctx.enter_context(tc.tile_pool(name="ipool", bufs=1))
            ps = ctx.enter_context(
                tc.tile_pool(name="ps", bufs=2, space="PSUM"))

            # DRAM tables / states / accumulators
            tabs = {}
            for r in range(c.rounds):
                tabs[("v2c", r)] = dram.tile([c.nvp, H], f32,
                                             name=f"tv{r}", tag=f"tv{r}")
                tabs[("c2v", r)] = dram.tile([c.ncp, H], f32,
                                             name=f"tc{r}", tag=f"tc{r}")
            accs = {}
            for r in range(c.rounds):
                accs[("v2c", r)] = [
                    dram.tile([c.cr, H], f32, name=f"acc_c{r}_{j}",
                              tag=f"acc_c{r}_{j}") for j in range(NACC)]
                accs[("c2v", r)] = [
                    dram.tile([c.vr, H], f32, name=f"acc_v{r}_{j}",
                              tag=f"acc_v{r}_{j}") for j in range(NACC)]
            chunk_var = [dram.tile([c.vr, H], f32, name=f"sv{j}", tag=f"sv{j}")
                         for j in range(2)]
            chunk_con = [dram.tile([c.cr, H], f32, name=f"sc{j}", tag=f"sc{j}")
                         for j in range(3)]

            ident = cpool.tile([128, 128], f32, name="ident", tag="ident")
            make_identity(nc, ident)

            # ---- zero the accumulators (overlaps encoder)
            zero_sb = cpool.tile([128, 2048], f32, name="zero_sb", tag="zsb")
            nc.vector.memset(zero_sb[:], 0.0)
            for (d, r), acc3 in (
                    {} if os.environ.get("BGNN_NOZERO") else accs).items():
                rows = c.cr if d == "v2c" else c.vr
                for acc in acc3:
                    # partition p owns a contiguous rows/128 block: one
                    # large contiguous descriptor per partition per strip
                    flat = acc[:, :].rearrange("(p x) f -> p (x f)", p=128)
                    per_p = rows * H // 128
                    o = 0
                    while o < per_p:
                        w = min(2048, per_p - o)
                        nc.sync.dma_start(flat[:, o:o + w],
                                          zero_sb[:, :w])
                        o += w

            # ---------------- encoder: x_t [F, rows] -> chunk [rows, H]
            def encode(x_t, F, rows, w1, b1, w2, b2, out_chunk):
                w1_sb = sb2.tile([F, H], f32, name="w1_sb", tag="encw1")
                nc.sync.dma_start(w1_sb[:], w1[:])
                w2_sb = sb2.tile([H, H], f32, name="w2_sb", tag="encw2")
                nc.sync.dma_start(w2_sb[:], w2[:])
                b1_sb = sb2.tile([H, 1], f32, name="b1_sb", tag="encb1")
                nc.sync.dma_start(b1_sb[:], b1[:])
                b2_sb = sb2.tile([H, 1], f32, name="b2_sb", tag="encb2")
                nc.sync.dma_start(b2_sb[:], b2[:])
                for t in range(rows // 512):
                    xt = sb.tile([F, 512], f32, name="xt", tag="enc_xt")
                    nc.sync.dma_start(xt[:], x_t[:, t * 512:(t + 1) * 512])
                    p1 = ps.tile([H, 512], f32, name="p1", tag="mmA")
                    nc.tensor.matmul(p1[:], lhsT=w1_sb[:], rhs=xt[:],
                                     start=True, stop=True)
                    t1 = sb.tile([H, 512], f32, name="t1", tag="enc_t1")
                    nc.scalar.activation(t1[:], p1[:],
                                         mybir.ActivationFunctionType.Tanh,
                                         bias=b1_sb[:, :])
                    p2 = ps.tile([H, 512], f32, name="p2", tag="mmA")
                    nc.tensor.matmul(p2[:], lhsT=w2_sb[:], rhs=t1[:],
                                     start=True, stop=True)
                    h2 = sb.tile([H, 512], f32, name="h2", tag="enc_h2")
                    nc.vector.tensor_scalar_add(h2[:], p2[:], b2_sb[:, :])
                    hn = sb.tile([128, 4 * H], f32, name="hn", tag="enc_hn")
                    for q in range(4):
                        pt = ps.tile([128, H], f32, name="pt", tag="trp")
                        nc.tensor.transpose(
                            pt[:], h2[:, q * 128:(q + 1) * 128], ident[:H, :H])
                        nc.scalar.activation(
                            hn[:, q * H:(q + 1) * H], pt[:],
                            mybir.ActivationFunctionType.Copy)
                    nc.sync.dma_start(
                        out_chunk[t * 512:(t + 1) * 512, :].rearrange(
                            "(q p) f -> p q f", p=128), hn[:])

            if trunc >= 0:
                encode(xv_t, c.vf, c.vr, w1v_t, b1v, w2v_t, b2v,
                       chunk_var[0])
                encode(xc_t, c.cf, c.cr, w1c_t, b1c, w2c_t, b2c,
                       chunk_con[0])

            def allgather(chunk, tab):
                nc.gpsimd.collective_compute(
                    "AllGather", mybir.AluOpType.bypass,
                    replica_groups=groups,
                    ins=[chunk[:, :]],
                    outs=[tab[:, :]],
                )

            if trunc >= 1:
                allgather(chunk_var[0], tabs[("v2c", 0)])

            # ---------------- one message-passing direction
            def msg_pass(d, r, tab, tab_rows, cells, acc3, rng, deg,
                         h_old_chunk, out_chunk, readout=None,
                         do_gs=True, do_hnew=True):
                w_t, b_rep = wts[(d, r)]
                E_dir = sum(sum(w) for w in cells.values())
                gidx, sidx = ((gidx_v2c, sidx_v2c) if d == "v2c"
                              else (gidx_c2v, sidx_c2v))
                wt_sb = sb2.tile([H, H], f32, name="wt_sb", tag="msg_wt")
                nc.sync.dma_start(wt_sb[:], w_t[:])
                brep_sb = sb2.tile([128, H], f32, name="brep_sb",
                                   tag="msg_brep")
                nc.sync.dma_start(brep_sb[:], b_rep[:])
                gi_sb = ipool.tile([128, E_dir // 16], i16, name="gi_sb",
                                   tag="gi_sb")
                nc.sync.dma_start(gi_sb[:], gidx[:])
                si_sb = ipool.tile([128, E_dir // 16], i16, name="si_sb",
                                   tag="si_sb")
                nc.sync.dma_start(si_sb[:], sidx[:])

                # gather/scatter cells, range-major so ranges finish in
                # order; scatters split into unique-dest waves rotated
                # across NACC accumulators (collisions within one scatter
                # instruction lose updates; across serialized or
                # different-acc instructions they accumulate correctly)
                offs = {}
                o = 0
                for key in sorted(cells):
                    offs[key] = o
                    o += sum(cells[key])
                # dma_gather is limited to <=1024 idxs per instruction
                # (SWDGE ring); larger cells split into batches filling one
                # gbuf.  Cells are software-pipelined: emit cell N's
                # gathers, then cell N-1's scatter waves, so scatter
                # serialization stalls overlap gather desc-gen.
                GB = 1024
                wc = 0
                pending = None  # (k, key, gbuf)
                cell_list = []
                for k in range(c.nrange if do_gs else 0):
                    for key in sorted(cells):
                        if key[0] == k:
                            cell_list.append((k, key))

                def emit_scatters(k, key, gbuf):
                    nonlocal wc
                    wo = 0
                    for wsz in cells[key]:
                        acc = acc3[wc % len(acc3)]
                        wc += 1
                        nc.gpsimd.dma_scatter_add(
                            acc[k * rng:(k + 1) * rng, :],
                            gbuf[:, wo // 128:(wo + wsz) // 128, :],
                            si_sb[:, (offs[key] + wo) // 16:
                                  (offs[key] + wo + wsz) // 16],
                            wsz, wsz, H)
                        wo += wsz

                for (k, key) in cell_list:
                    ch = key[1]
                    sz = sum(cells[key])
                    gbuf = gpool.tile([128, B // 128, H], f32,
                                      name="gbuf", tag="gbuf")
                    src_ap = tab[ch * c.chunk:
                                 min((ch + 1) * c.chunk, tab_rows), :]
                    go = 0
                    while go < sz:
                        gsz = min(GB, sz - go)
                        o16 = (offs[key] + go) // 16
                        nc.gpsimd.dma_gather(
                            gbuf[:, go // 128:(go + gsz) // 128, :],
                            src_ap, gi_sb[:, o16:o16 + gsz // 16],
                            gsz, gsz, H)
                        go += gsz
                    if pending is not None:
                        emit_scatters(*pending)
                    pending = (k, key, gbuf)
                if pending is not None:
                    emit_scatters(*pending)

                # per-range h_new pipeline (512-row groups of <=4 tiles)
                ntile = (c.cr if d == "v2c" else c.vr) // 128
                rtile = rng // 128
                for k in range(c.nrange if do_hnew else 0):
                    t0 = k * rtile
                    while t0 < (k + 1) * rtile:
                        g = min(4, (k + 1) * rtile - t0)
                        a_sb = sb.tile([128, 4, H], f32, name="a_sb",
                                       tag="a_sb")
                        nc.sync.dma_start(
                            a_sb[:, :g, :],
                            acc3[0][t0 * 128:(t0 + g) * 128, :].rearrange(
                                "(q p) f -> p q f", p=128))
                        for j in range(1, len(acc3)):
                            aj = sb.tile([128, 4, H], f32, name="aj",
                                         tag=f"aj{j}")
                            nc.sync.dma_start(
                                aj[:, :g, :],
                                acc3[j][t0 * 128:(t0 + g) * 128, :]
                                .rearrange("(q p) f -> p q f", p=128))
                            nc.vector.tensor_add(
                                a_sb[:, :g, :], a_sb[:, :g, :], aj[:, :g, :])
                        hold = sb.tile([128, 4, H], f32, name="hold",
                                       tag="hold")
                        nc.sync.dma_start(
                            hold[:, :g, :],
                            h_old_chunk[t0 * 128:(t0 + g) * 128, :].rearrange(
                                "(q p) f -> p q f", p=128))
                        degc = sb.tile([128, 4], f32, name="degc", tag="degc")
                        nc.sync.dma_start(
                            degc[:, :g],
                            deg[t0 * 128:(t0 + g) * 128].rearrange(
                                "(q p) -> p q", p=128))
                        hnew = sb.tile([128, 4, H], f32, name="hnew",
                                       tag="hnew")
                        for q in range(g):
                            ptr = ps.tile([H, 128], f32, name="ptr", tag="trp")
                            nc.tensor.transpose(ptr[:], a_sb[:, q, :],
                                                ident[:, :])
                            afm = sb.tile([H, 128], f32, name="afm", tag="afm")
                            nc.scalar.activation(
                                afm[:], ptr[:],
                                mybir.ActivationFunctionType.Copy)
                            agg = ps.tile([128, H], f32, name="agg", tag="agg")
                            nc.tensor.matmul(agg[:], lhsT=afm[:], rhs=wt_sb[:],
                                             start=True, stop=True)
                            hk = hnew[:, q, :]
                            nc.vector.tensor_scalar_mul(
                                hk, brep_sb[:], degc[:, q:q + 1])
                            nc.vector.tensor_add(hk, hk, hold[:, q, :])
                            nc.vector.tensor_add(hk, hk, agg[:])
                            nc.scalar.activation(
                                hk, hk, mybir.ActivationFunctionType.Tanh)
                        if out_chunk is not None:
                            nc.sync.dma_start(
                                out_chunk[t0 * 128:(t0 + g) * 128, :]
                                .rearrange("(q p) f -> p q f", p=128),
                                hnew[:, :g, :])
                        if readout is not None:
                            m = sb.tile([128, 4, H], f32, name="m", tag="rom")
                            nc.vector.tensor_mul(
                                m[:, :g, :], hnew[:, :g, :],
                                wro4_sb[:].rearrange(
                                    "p (q f) -> p q f", q=4)[:, :g, :])
                            nc.vector.tensor_reduce(
                                readout[:, t0:t0 + g],
                                m[:, :g, :], axis=mybir.AxisListType.X,
                                op=mybir.AluOpType.add)
                        t0 += g

            wro4_sb = cpool.tile([128, 4 * H], f32, name="wro4_sb",
                                 tag="wro4_sb")
            for q in range(4):
                nc.sync.dma_start(wro4_sb[:, q * H:(q + 1) * H], wro_rep[:])
            sc_sb = cpool.tile([128, c.vr // 128], f32, name="sc_sb",
                               tag="sc_sb")
            nc.vector.memset(sc_sb[:], 0.0)

            con_state = chunk_con[0]
            var_state = chunk_var[0]
            seq = []
            for r in range(c.rounds):
                seq.append(("v2c", r))
                seq.append(("c2v", r))
            for di, (d, r) in enumerate(seq):
                last = (d, r) == seq[-1]
                do_gs = trunc >= 2 + 2 * di
                do_hnew = trunc >= 3 + 2 * di
                if not do_gs:
                    break
                if d == "v2c":
                    out = chunk_con[r + 1]
                    msg_pass(d, r, tabs[("v2c", r)], c.nvp, cells_v2c,
                             accs[("v2c", r)], c.crange, deg_con,
                             con_state, out, do_gs=do_gs, do_hnew=do_hnew)
                    if do_hnew:
                        allgather(out, tabs[("c2v", r)])
                    con_state = out
                else:
                    out = None if last else chunk_var[r + 1]
                    msg_pass(d, r, tabs[("c2v", r)], c.ncp, cells_c2v,
                             accs[("c2v", r)], c.vrange, deg_var,
                             var_state, out,
                             readout=sc_sb if last else None,
                             do_gs=do_gs, do_hnew=do_hnew)
                    if not last and do_hnew:
                        allgather(out, tabs[("v2c", r + 1)])
                        var_state = out

            # readout epilogue: sc_sb [128, ntiles] -> scores [vr]
            nt = c.vr // 128
            nc.vector.tensor_scalar_add(sc_sb[:], sc_sb[:], float(b_ro))
            for half in range(2):
                w2 = nt // 2
                pt = ps.tile([w2, 128], f32, name="pt_ro", tag="trp")
                nc.tensor.transpose(
                    pt[:], sc_sb[:, half * w2:(half + 1) * w2], ident[:, :])
                so = sb.tile([w2, 128], f32, name="so", tag="so")
                nc.scalar.activation(
                    so[:], pt[:], mybir.ActivationFunctionType.Copy)
                nc.sync.dma_start(
                    scores[half * w2 * 128:(half + 1) * w2 * 128].rearrange(
                        "(q p) -> q p", p=128), so[:])

    nc.compile()
    return nc


# ---------------------------------------------------------------- runner

_CACHE = {}


def _get_nc(cfg, meta):
    import os
    key = (cfg, meta, os.environ.get("BGNN_TRUNC", "99"))
    if key not in _CACHE:
        _CACHE[key] = build_nc(cfg, meta)
    return _CACHE[key]


def run(inputs, cfg: Cfg = FULL, trace=False):
    from concourse import bass_utils
    per_core, meta = prep_inputs(inputs, cfg)
    nc = _get_nc(cfg, meta)
    res = bass_utils.run_bass_kernel_spmd(
        nc, per_core, core_ids=list(range(cfg.n_cores)), trace=trace)
    return extract_scores(res.results, cfg), res


def kernel(**inputs) -> np.ndarray:
    out, _ = run(inputs, FULL)
    return out
